# revision 54
# baseline (speedup 1.0000x reference)
"""Trainium2 Bass kernel for nn_AMIPRouterInference (gnn_message_passing).

Strategy (8 NeuronCores, expert-parallel):
  - Each core owns one of the K=8 experts (weights read from HBM exactly once
    chip-wide).  The router / q / k weights are replicated (tiny).
  - The first MLP layer is deduplicated: instead of computing
    gelu(cond @ W1) per (mask, anchor) pair (10x redundant), we compute
    A1T once per anchor row and M1T once per mask row, then combine shifted
    planes.  All tensors live in a transposed [feature-partition,
    position-free] layout so the (anchor - mask) index offsets become
    free-axis shifts.
  - Pair combine weights (segment softmax * router gate) are computed as
    partition-0 rows via ones-vector column-sum matmuls, then broadcast to
    128 partitions with K=1 matmuls and folded into the plane accumulation,
    which shrinks the second MLP matmul by 10x as well.
  - Each core produces a full [D, M] partial delta (its expert, all masks);
    a ReduceScatter over the 8 cores sums the experts and leaves each core
    with a [D/8, M] slice, DMA'd out.  The host reassembles / transposes and
    scatters rows into the [1, L, D] output.

The pair tables (which (mask, anchor) pairs exist) are integer-only host
work derived from the runtime index inputs; they parameterize the compiled
graph (offset planes + validity masks).
"""

import os
import numpy as np

NCORES = 8

_GRAPH_CACHE = {}
LAST_RESULT = None  # BassKernelResults of the most recent device run


# ----------------------------------------------------------------------------
# Host-side pair-table construction (mirrors reference semantics exactly)
# ----------------------------------------------------------------------------

def build_tables(m_idx, u_idx, r, pmax):
    M = len(m_idx)
    dists = np.abs(m_idx[:, None].astype(np.int64) - u_idx[None, :].astype(np.int64))
    adj = (dists > 0) & (dists <= r)
    pair_m, pair_u = np.nonzero(adj)  # row-major == jnp.nonzero order
    pair_m = pair_m[:pmax]
    pair_u = pair_u[:pmax]
    offs = np.unique(pair_u - pair_m).astype(np.int64)
    J = len(offs)
    valid = np.zeros((J, M), dtype=np.float32)
    for j, d in enumerate(offs):
        valid[j, pair_m[(pair_u - pair_m) == d]] = 1.0
    return offs, valid


# ----------------------------------------------------------------------------
# Graph builder (SPMD: all cores run this graph with different input data)
# ----------------------------------------------------------------------------

def build_graph(cfg):
    import contextlib
    import concourse.mybir as mybir
    import concourse.tile as tile
    from concourse import bacc

    D, H, M, U, DP, K = cfg["D"], cfg["H"], cfg["M"], cfg["U"], cfg["DP"], cfg["K"]
    NC = cfg["NC"]
    offs = cfg["offs"]
    J = len(offs)
    PAD = cfg["PAD"]
    MCW = cfg["MCW"]            # compute chunk width along M
    NMC = M // MCW
    QCW = cfg["QCW"]            # qk/score-phase chunk width
    NQC = M // QCW
    DB, HB, DPB = D // 128, H // 128, DP // 128
    HGS = min(4, HB)            # h-blocks per A/M-phase psum group
    DGS = min(4, DB)            # d-blocks per W2-phase psum group
    RSD = D // NC               # rows of final output per core
    NRS = cfg["NRS"]            # number of reduce-scatter column groups
    RSW = M // NRS
    assert M % MCW == 0 and M % QCW == 0 and M % NRS == 0 and (RSW % MCW == 0)

    bf16 = mybir.dt.bfloat16
    f32 = mybir.dt.float32
    AF = mybir.ActivationFunctionType
    hid_af = getattr(AF, cfg.get("hid_act", "Gelu"))

    nc = bacc.Bacc(None, target_bir_lowering=False, debug=False)

    # ---------------- DRAM parameters ----------------
    hmT = nc.declare_dram_parameter("hmT", [D, M], bf16, isOutput=False)
    huT = nc.declare_dram_parameter("huT", [D, U], bf16, isOutput=False)
    w1a = nc.declare_dram_parameter("w1a", [D, H], bf16, isOutput=False)
    w1b = nc.declare_dram_parameter("w1b", [D, H], bf16, isOutput=False)
    w2 = nc.declare_dram_parameter("w2", [H, D], bf16, isOutput=False)
    wq = nc.declare_dram_parameter("wq", [D, DP], bf16, isOutput=False)
    wk = nc.declare_dram_parameter("wk", [D, DP], bf16, isOutput=False)
    wr = nc.declare_dram_parameter("wr", [D, K], bf16, isOutput=False)
    b1c = nc.declare_dram_parameter("b1c", [128, HB], f32, isOutput=False)
    b2r = nc.declare_dram_parameter("b2r", [1, D], bf16, isOutput=False)
    bqc = nc.declare_dram_parameter("bqc", [128, DPB], f32, isOutput=False)
    bkc = nc.declare_dram_parameter("bkc", [128, DPB], f32, isOutput=False)
    brc = nc.declare_dram_parameter("brc", [128, 1], f32, isOutput=False)
    esel = nc.declare_dram_parameter("esel", [K, 1], f32, isOutput=False)
    vmask = nc.declare_dram_parameter("vmask", [J, M], bf16, isOutput=False)
    outp = nc.declare_dram_parameter("out", [RSD, M], bf16, isOutput=True)

    with tile.TileContext(nc) as tc, contextlib.ExitStack() as ctx:
        sb = ctx.enter_context(tc.tile_pool(name="sb", bufs=1))
        ps = ctx.enter_context(tc.tile_pool(name="ps", bufs=1, space="PSUM"))
        dram = ctx.enter_context(tc.tile_pool(name="dram", bufs=1, space="DRAM"))

        def psum_mm(name):
            return ps.tile([128, 512], f32, tag="mm", bufs=6, name=name)

        def psum_row(name):
            return ps.tile([8, 512], f32, tag="row", bufs=2, name=name)

        # ---------------- persistent SBUF tensors ----------------
        ones = sb.tile([128, 128], bf16, name="ones")
        nc.vector.memset(ones[:, :], 1.0)
        ones32 = sb.tile([128, 1], f32, name="ones32")
        nc.vector.memset(ones32[:, :], 1.0)

        b1_sb = sb.tile([128, HB], f32, name="b1_sb")
        nc.sync.dma_start(b1_sb[:, :], b1c[:, :])
        bq_sb = sb.tile([128, DPB], f32, name="bq_sb")
        nc.sync.dma_start(bq_sb[:, :], bqc[:, :])
        bk_sb = sb.tile([128, DPB], f32, name="bk_sb")
        nc.sync.dma_start(bk_sb[:, :], bkc[:, :])
        br_sb = sb.tile([128, 1], f32, name="br_sb")
        nc.sync.dma_start(br_sb[:, :], brc[:, :])
        esel_sb = sb.tile([K, 1], f32, name="esel_sb")
        nc.sync.dma_start(esel_sb[:, :], esel[:, :])
        wr_sb = sb.tile([128, DB, K], bf16, name="wr_sb")
        nc.sync.dma_start(
            wr_sb[:, :, :], wr.ap().rearrange("(o p) k -> p o k", p=128)
        )
        b2_sb = sb.tile([1, D], bf16, name="b2_sb")
        nc.sync.dma_start(b2_sb[:, :], b2r[:, :])

        # steady-phase streaming tiles live in the persistent pool so their
        # addresses never overlap the phase-1/2 pool (overlap deps would
        # serialize the first A-phase against the score computation)
        def hmc_tile(name):
            return sb.tile([128, D // 128, cfg["MCW"]], bf16, tag="hmc",
                           bufs=2, name=name)

        def w1t_tile(name):
            return sb.tile([128, 4, min(4, H // 128) * 128], bf16, tag="w1t",
                           bufs=2, name=name)

        def w2t_tile(name):
            return sb.tile([128, 4, min(4, D // 128) * 128], bf16, tag="w2t",
                           bufs=2, name=name)

        A1T = sb.tile([128, HB, U + 2 * PAD], bf16, name="A1T")
        for hb in range(HB):
            nc.vector.memset(A1T[:, hb, 0:PAD], 0.0)
            nc.vector.memset(A1T[:, hb, PAD + U: U + 2 * PAD], 0.0)

        # combine-weight rows live in DRAM between phase 2 and the hid phases
        cw_dram = dram.tile([J, M], bf16, name="cw_dram")
        cwsum_bf = sb.tile([1, M], bf16, name="cwsum_bf")

        # ================= phases 1+2 (scoped pool) =================
        with tc.tile_pool(name="p12", bufs=1) as p12:
            kT_sb = p12.tile([128, DPB, U + 2 * PAD], bf16, name="kT_sb")
            for dpb in range(DPB):
                nc.vector.memset(kT_sb[:, dpb, 0:PAD], 0.0)
                nc.vector.memset(kT_sb[:, dpb, PAD + U: U + 2 * PAD], 0.0)
            qT_sb = p12.tile([128, DPB, M], bf16, name="qT_sb")
            eg_sb = p12.tile([K, M], f32, name="eg_sb")

            # ---- phase 1: q / k / gate matmuls ----
            DBB = min(4, DB)    # d-blocks fetched per DMA
            for ch in range(NQC):
                csl = slice(ch * QCW, (ch + 1) * QCW)
                # q + gate sweep (rhs: hmT tiles streamed, batched fetches)
                psq = [psum_mm(f"psq{i}") for i in range(DPB)]
                psg = psum_row("psg")
                for dbb in range(0, DB, DBB):
                    hm_t = p12.tile([128, DBB, QCW], bf16, tag="ht", bufs=2,
                                    name="hm_t")
                    nc.sync.dma_start(
                        hm_t[:, :, :],
                        hmT[dbb * 128:(dbb + DBB) * 128, csl].rearrange(
                            "(o p) m -> p o m", p=128),
                    )
                    wq_t = p12.tile([128, DBB, DP], bf16, tag="wt", bufs=2,
                                    name="wq_t")
                    nc.sync.dma_start(
                        wq_t[:, :, :],
                        wq[dbb * 128:(dbb + DBB) * 128, :].rearrange(
                            "(o p) m -> p o m", p=128),
                    )
                    for i in range(DBB):
                        db = dbb + i
                        st, sp = db == 0, db == DB - 1
                        for dpb in range(DPB):
                            nc.tensor.matmul(
                                psq[dpb][:, :QCW],
                                wq_t[:, i, dpb * 128:(dpb + 1) * 128],
                                hm_t[:, i, :],
                                start=st, stop=sp,
                            )
                        nc.tensor.matmul(
                            psg[:K, :QCW], wr_sb[:, db, :], hm_t[:, i, :],
                            start=st, stop=sp,
                        )
                for dpb in range(DPB):
                    nc.vector.tensor_scalar_add(
                        qT_sb[:, dpb, csl], psq[dpb][:, :QCW],
                        bq_sb[:, dpb:dpb + 1],
                    )
                nc.scalar.activation(
                    eg_sb[:, csl], psg[:K, :QCW], AF.Exp,
                    bias=br_sb[0:K, 0:1], scale=1.0,
                )
                # k sweep
                psk = [psum_mm(f"psk{i}") for i in range(DPB)]
                for dbb in range(0, DB, DBB):
                    hu_t = p12.tile([128, DBB, QCW], bf16, tag="ht", bufs=2,
                                    name="hu_t")
                    nc.sync.dma_start(
                        hu_t[:, :, :],
                        huT[dbb * 128:(dbb + DBB) * 128, csl].rearrange(
                            "(o p) m -> p o m", p=128),
                    )
                    wk_t = p12.tile([128, DBB, DP], bf16, tag="wt", bufs=2,
                                    name="wk_t")
                    nc.sync.dma_start(
                        wk_t[:, :, :],
                        wk[dbb * 128:(dbb + DBB) * 128, :].rearrange(
                            "(o p) m -> p o m", p=128),
                    )
                    for i in range(DBB):
                        db = dbb + i
                        for dpb in range(DPB):
                            nc.tensor.matmul(
                                psk[dpb][:, :QCW],
                                wk_t[:, i, dpb * 128:(dpb + 1) * 128],
                                hu_t[:, i, :],
                                start=db == 0, stop=db == DB - 1,
                            )
                for dpb in range(DPB):
                    nc.vector.tensor_scalar_add(
                        kT_sb[:, dpb, PAD + ch * QCW: PAD + (ch + 1) * QCW],
                        psk[dpb][:, :QCW], bk_sb[:, dpb:dpb + 1],
                    )

            # ---- phase 2: scores / softmax / gate / combine rows ----
            inv_sqrt_dp = 1.0 / float(np.sqrt(DP))
            P2W = min(QCW, 256)
            for ch in range(M // P2W):
                csl2 = slice(ch * P2W, (ch + 1) * P2W)
                ej = p12.tile([1, J * P2W], f32, tag="ej", bufs=1, name="ej")
                vm_ch = p12.tile([1, J * P2W], bf16, tag="vm", bufs=1,
                                 name="vm_ch")
                for j in range(J):
                    nc.sync.dma_start(
                        vm_ch[0:1, j * P2W:(j + 1) * P2W], vmask[j:j + 1, csl2]
                    )
                rowt = p12.tile([1, 6 * P2W], f32, tag="rw", bufs=1, name="rowt")

                def rw(i):
                    return rowt[0:1, i * P2W:(i + 1) * P2W]

                for j in range(J):
                    off = int(offs[j])
                    pss = psum_row(f"pss{j}")
                    for dpb in range(DPB):
                        prod = p12.tile([128, P2W], bf16, tag="prod", bufs=2,
                                        name="prod")
                        nc.vector.tensor_mul(
                            out=prod[:, :],
                            in0=qT_sb[:, dpb, csl2],
                            in1=kT_sb[:, dpb,
                                      PAD + off + ch * P2W:
                                      PAD + off + (ch + 1) * P2W],
                        )
                        nc.tensor.matmul(
                            pss[0:1, :P2W], ones[:, 0:1], prod[:, :],
                            start=dpb == 0, stop=dpb == DPB - 1,
                        )
                    ejr = ej[0:1, j * P2W:(j + 1) * P2W]
                    nc.scalar.activation(
                        ejr, pss[0:1, :P2W], AF.Exp, bias=0.0,
                        scale=inv_sqrt_dp,
                    )
                    nc.vector.tensor_mul(
                        out=ejr, in0=ejr,
                        in1=vm_ch[0:1, j * P2W:(j + 1) * P2W],
                    )
                # ssum / srecip
                nc.vector.tensor_add(
                    out=rw(0), in0=ej[0:1, 0:P2W], in1=ej[0:1, P2W:2 * P2W]
                )
                for j in range(2, J):
                    nc.vector.tensor_add(
                        out=rw(0), in0=rw(0),
                        in1=ej[0:1, j * P2W:(j + 1) * P2W],
                    )
                nc.vector.tensor_scalar_max(rw(1), rw(0), 1e-8)
                nc.vector.reciprocal(rw(1), rw(1))
                # gate: gsum over K partitions, recip, selected expert
                psr = psum_row("psgs")
                nc.tensor.matmul(
                    psr[0:1, :P2W], ones32[0:K, 0:1], eg_sb[:, csl2],
                    start=True, stop=True,
                )
                nc.vector.tensor_copy(rw(2), psr[0:1, :P2W])
                nc.vector.reciprocal(rw(2), rw(2))
                pse = psum_row("pse")
                nc.tensor.matmul(
                    pse[0:1, :P2W], esel_sb[:, 0:1], eg_sb[:, csl2],
                    start=True, stop=True,
                )
                nc.vector.tensor_copy(rw(3), pse[0:1, :P2W])
                # ge = sel * grecip ; rg = srecip * ge
                nc.vector.tensor_mul(out=rw(3), in0=rw(3), in1=rw(2))
                nc.vector.tensor_mul(out=rw(4), in0=rw(1), in1=rw(3))
                # cw_j = ej * rg ; cwsum = ssum * rg
                cwrow = p12.tile([1, J * P2W], bf16, tag="cwr", bufs=1,
                                 name="cwrow")
                for j in range(J):
                    nc.vector.tensor_mul(
                        out=cwrow[0:1, j * P2W:(j + 1) * P2W],
                        in0=ej[0:1, j * P2W:(j + 1) * P2W], in1=rw(4),
                    )
                for j in range(J):
                    nc.sync.dma_start(
                        cw_dram[j:j + 1, csl2],
                        cwrow[0:1, j * P2W:(j + 1) * P2W],
                    )
                nc.vector.tensor_mul(out=rw(5), in0=rw(0), in1=rw(4))
                nc.vector.tensor_copy(cwsum_bf[0:1, csl2], rw(5))

        # ================= steady phases (scoped pools) =================
        SCW = cfg.get("SCW", MCW)       # hid-phase sub-chunk width
        NSC = MCW // SCW
        with tc.tile_pool(name="hresp", bufs=1) as hres, \
             tc.tile_pool(name="streamp", bufs=1) as stream, \
             tc.tile_pool(name="tmpp", bufs=1) as tmp:

            def msl(mc):
                return slice(mc * MCW, (mc + 1) * MCW)

            HGRP = [list(range(g, min(g + HGS, HB))) for g in range(0, HB, HGS)]
            DGRP = [list(range(g, min(g + DGS, DB))) for g in range(0, DB, DGS)]

            bounce = [
                dram.tile([D, RSW], bf16, name=f"bounce{g}") for g in range(NRS)
            ]
            rsout = [
                dram.tile([RSD, RSW], bf16, name=f"rsout{g}") for g in range(NRS)
            ]

            def a_or_m_phase(mc, which, m1b_t=None):
                src, wsrc = (huT, w1a) if which == "a" else (hmT, w1b)
                h_mc = hmc_tile(f"h_{which}{mc}")
                nc.sync.dma_start(
                    h_mc[:, :, :],
                    src.ap().rearrange("(o p) m -> p o m", p=128)[:, :, msl(mc)],
                )
                DBB = min(4, DB)
                for grp in HGRP:
                    gw = len(grp) * 128
                    psa = [psum_mm(f"ps{which}{i}") for i in range(len(grp))]
                    for dbb in range(0, DB, DBB):
                        w_t = w1t_tile("w1_t")
                        nc.sync.dma_start(
                            w_t[:, :DBB, :gw],
                            wsrc[dbb * 128:(dbb + DBB) * 128,
                                 grp[0] * 128: grp[0] * 128 + gw].rearrange(
                                     "(o p) h -> p o h", p=128),
                        )
                        for i in range(DBB):
                            db = dbb + i
                            for gi, hb in enumerate(grp):
                                nc.tensor.matmul(
                                    psa[gi][:, :MCW],
                                    w_t[:, i, gi * 128:(gi + 1) * 128],
                                    h_mc[:, db, :],
                                    start=db == 0, stop=db == DB - 1,
                                )
                    # psum -> SBUF casts on ScalarE (DVE is the busy engine)
                    for gi, hb in enumerate(grp):
                        if which == "a":
                            nc.scalar.activation(
                                A1T[:, hb, PAD + mc * MCW: PAD + (mc + 1) * MCW],
                                psa[gi][:, :MCW], AF.Identity,
                                bias=0.0, scale=1.0,
                            )
                        else:
                            for s in range(NSC):
                                nc.scalar.activation(
                                    m1b_t[s][:, hb, :],
                                    psa[gi][:, s * SCW:(s + 1) * SCW],
                                    AF.Identity,
                                    bias=b1_sb[:, hb:hb + 1], scale=1.0,
                                )

            def cwb_build(mc):
                # broadcast combine-weight rows to 128 partitions, early so
                # the K=1 matmuls slot into the A-phase and hid never waits
                cwsl = tmp.tile([1, J * MCW], bf16, tag="cwsl", bufs=1,
                                name="cwsl")
                for j in range(J):
                    nc.sync.dma_start(
                        cwsl[0:1, j * MCW:(j + 1) * MCW],
                        cw_dram[j:j + 1, msl(mc)],
                    )
                cwbs = []
                for s in range(NSC):
                    cwb = tmp.tile([128, J, SCW], bf16, tag="cwb", bufs=3,
                                   name=f"cwb{mc}_{s}")
                    for j in range(J):
                        psb = psum_mm(f"psb{j}")
                        nc.tensor.matmul(
                            psb[:, :SCW], ones[0:1, :],
                            cwsl[0:1, j * MCW + s * SCW: j * MCW + (s + 1) * SCW],
                            start=True, stop=True,
                        )
                        nc.vector.tensor_copy(cwb[:, j, :], psb[:, :SCW])
                    cwbs.append(cwb)
                return cwbs

            def hid_sub(mc, s, m1b_t, hid_t, cwb):
                c0 = mc * MCW + s * SCW
                for hb in range(HB):
                    for j in range(J):
                        off = int(offs[j])
                        x_t = tmp.tile([128, SCW], bf16, tag="xt", bufs=2,
                                       name="x_t")
                        nc.vector.tensor_add(
                            out=x_t[:, :],
                            in0=A1T[:, hb, PAD + off + c0: PAD + off + c0 + SCW],
                            in1=m1b_t[:, hb, :],
                        )
                        g_t = tmp.tile([128, SCW], bf16, tag="gt", bufs=2,
                                       name="g_t")
                        nc.scalar.activation(
                            g_t[:, :], x_t[:, :], hid_af, bias=0.0, scale=1.0,
                        )
                        dst = hid_t[:, hb, :]
                        if j == 0:
                            nc.vector.tensor_mul(
                                out=dst, in0=g_t[:, :], in1=cwb[:, j, :],
                            )
                        else:
                            nc.vector.tensor_mul(
                                out=g_t[:, :], in0=g_t[:, :], in1=cwb[:, j, :]
                            )
                            nc.vector.tensor_add(out=dst, in0=dst, in1=g_t[:, :])

            def w2_sub(mc, s, hid_t):
                c0 = mc * MCW + s * SCW
                g = c0 // RSW
                col0 = c0 - g * RSW
                HBB = min(4, HB)
                for grp in DGRP:
                    gw = len(grp) * 128
                    psd = [psum_mm(f"psd{i}") for i in range(len(grp))]
                    for hbb in range(0, HB, HBB):
                        w2_t = w2t_tile("w2_t")
                        nc.sync.dma_start(
                            w2_t[:, :HBB, :gw],
                            w2[hbb * 128:(hbb + HBB) * 128,
                               grp[0] * 128: grp[0] * 128 + gw].rearrange(
                                   "(o p) d -> p o d", p=128),
                        )
                        for i in range(HBB):
                            hb = hbb + i
                            for gi, db in enumerate(grp):
                                nc.tensor.matmul(
                                    psd[gi][:, :SCW],
                                    w2_t[:, i, gi * 128:(gi + 1) * 128],
                                    hid_t[:, hb, :],
                                    start=hb == 0, stop=False,
                                )
                    for gi, db in enumerate(grp):
                        nc.tensor.matmul(
                            psd[gi][:, :SCW],
                            b2_sb[0:1, db * 128:(db + 1) * 128],
                            cwsum_bf[0:1, c0:c0 + SCW],
                            start=False, stop=True,
                        )
                        d_t = tmp.tile([128, SCW], bf16, tag="dt", bufs=2,
                                       name="d_t")
                        # psum -> SBUF on ScalarE: keeps DVE free for hid work
                        nc.scalar.activation(
                            d_t[:, :], psd[gi][:, :SCW], AF.Identity,
                            bias=0.0, scale=1.0,
                        )
                        nc.sync.dma_start(
                            bounce[g][db * 128:(db + 1) * 128,
                                      col0:col0 + SCW],
                            d_t[:, :],
                        )

            def rs_maybe(mc):
                if (mc + 1) % max(1, RSW // MCW) == 0:
                    g = (mc * MCW) // RSW
                    nc.gpsimd.collective_compute(
                        "ReduceScatter",
                        mybir.AluOpType.add,
                        ins=[bounce[g].opt()],
                        outs=[rsout[g].opt()],
                        replica_groups=[list(range(NC))],
                    )
                    nc.sync.dma_start(
                        outp[:, g * RSW:(g + 1) * RSW], rsout[g][:, :]
                    )

            m1b_store, hid_store, cwb_store = {}, {}, {}

            def new_hid_tile(mc, s):
                return tmp.tile([128, HB, SCW], bf16, tag="hidct", bufs=3,
                                name=f"hid_{mc}_{s}")

            for mc in range(NMC):
                a_or_m_phase(mc, "a")
                cwb_store[mc] = cwb_build(mc)
                m1b_store[mc] = [
                    tmp.tile([128, HB, SCW], bf16, tag="m1b", bufs=3,
                             name=f"m1b_{mc}_{s}")
                    for s in range(NSC)
                ]
                a_or_m_phase(mc, "m", m1b_store[mc])
                if mc >= 1:
                    # previous chunk's last sub-chunk needed this A's halo
                    pm, ls = mc - 1, NSC - 1
                    ht = new_hid_tile(pm, ls)
                    hid_sub(pm, ls, m1b_store[pm][ls], ht, cwb_store[pm][ls])
                    w2_sub(pm, ls, ht)
                    rs_maybe(pm)
                    del m1b_store[pm], cwb_store[pm]
                for s in range(NSC - 1):
                    ht = new_hid_tile(mc, s)
                    hid_sub(mc, s, m1b_store[mc][s], ht, cwb_store[mc][s])
                    w2_sub(mc, s, ht)
            mc, ls = NMC - 1, NSC - 1
            ht = new_hid_tile(mc, ls)
            hid_sub(mc, ls, m1b_store[mc][ls], ht, cwb_store[mc][ls])
            w2_sub(mc, ls, ht)
            rs_maybe(mc)

    nc.finalize()
    return nc


# ----------------------------------------------------------------------------
# Host wrapper
# ----------------------------------------------------------------------------

def _prepare(inputs, cfg):
    import ml_dtypes
    BF16 = ml_dtypes.bfloat16
    D, H, M, U, DP, K = cfg["D"], cfg["H"], cfg["M"], cfg["U"], cfg["DP"], cfg["K"]
    HB, DPB = H // 128, DP // 128
    offs, valid = cfg["offs"], cfg["valid"]
    J = len(offs)
    QCW = cfg["QCW"]
    NQC = M // QCW

    h = np.asarray(inputs["h_L"], dtype=np.float32)[0]
    m_idx = np.asarray(inputs["mask_indices"]).astype(np.int64)
    u_idx = np.asarray(inputs["unmasked_indices"]).astype(np.int64)

    hmT = np.ascontiguousarray(h[m_idx].astype(BF16).T)
    huT = np.ascontiguousarray(h[u_idx].astype(BF16).T)
    wq = np.asarray(inputs["Wq"], np.float32).astype(BF16)
    wk = np.asarray(inputs["Wk"], np.float32).astype(BF16)
    wr = np.asarray(inputs["Wr"], np.float32).astype(BF16)
    bqc = np.ascontiguousarray(
        np.asarray(inputs["bq"], np.float32).reshape(DPB, 128).T)
    bkc = np.ascontiguousarray(
        np.asarray(inputs["bk"], np.float32).reshape(DPB, 128).T)
    brc = np.zeros((128, 1), np.float32)
    brc[:K, 0] = np.asarray(inputs["br"], np.float32)
    vm = np.ascontiguousarray(valid).astype(BF16)  # [J, M]

    W1 = np.asarray(inputs["W1"], np.float32)
    W2 = np.asarray(inputs["W2"], np.float32)
    b1 = np.asarray(inputs["b1"], np.float32)
    b2 = np.asarray(inputs["b2"], np.float32)

    in_maps = []
    for c in range(cfg["NC"]):
        e = c % K
        sel = np.zeros((K, 1), np.float32)
        sel[e, 0] = 1.0
        in_maps.append({
            "hmT": hmT, "huT": huT,
            "w1a": np.ascontiguousarray(W1[e][:D]).astype(BF16),
            "w1b": np.ascontiguousarray(W1[e][D:]).astype(BF16),
            "w2": W2[e].astype(BF16),
            "wq": wq, "wk": wk, "wr": wr,
            "b1c": np.ascontiguousarray(b1[e].reshape(HB, 128).T),
            "b2r": b2[e].reshape(1, D).astype(BF16),
            "bqc": bqc, "bkc": bkc, "brc": brc,
            "esel": sel, "vmask": vm,
        })
    return in_maps, m_idx


def _run(cfg, in_maps, trace=False, sim=False):
    global LAST_RESULT
    key = cfg["key"]
    if key not in _GRAPH_CACHE:
        _GRAPH_CACHE[key] = build_graph(cfg)
    nc = _GRAPH_CACHE[key]
    if sim:
        from concourse import bass_interp
        s = bass_interp.MultiCoreSim(nc, cfg["NC"])
        for c in range(cfg["NC"]):
            for k, v in in_maps[c].items():
                s.cores[c].tensor(k)[:] = v
        s.simulate(check_with_hw=False)
        return [{"out": np.asarray(s.cores[c].mem_tensor("out"))}
                for c in range(cfg["NC"])]
    from concourse import bass_utils
    kw = {}
    if trace and os.environ.get("KERNEL_TRACE_DIR"):
        kw["tmpdir"] = os.environ["KERNEL_TRACE_DIR"]
    res = bass_utils.run_bass_kernel_spmd(
        nc, in_maps, core_ids=list(range(cfg["NC"])), trace=trace, **kw,
    )
    LAST_RESULT = res
    return res.results


def kernel_impl(inputs, D, K, L, M, U, DP, H, NC, MCW, QCW, NRS, sim=False,
                hid_act="Gelu", SCW=None):
    PMAX = M * 10

    m_idx = np.asarray(inputs["mask_indices"]).astype(np.int64)
    u_idx = np.asarray(inputs["unmasked_indices"]).astype(np.int64)
    r = int(np.asarray(inputs["range_r"]))

    offs, valid = build_tables(m_idx, u_idx, r, PMAX)
    J = len(offs)
    if J == 0:
        return np.zeros((1, L, D), np.float32)
    PAD = int(max(8, np.max(np.abs(offs))))
    PAD = (PAD + 7) // 8 * 8

    cfg = {
        "D": D, "H": H, "M": M, "U": U, "DP": DP, "K": K, "NC": NC,
        "offs": offs, "valid": valid, "PAD": PAD,
        "MCW": MCW, "QCW": QCW, "NRS": NRS, "hid_act": hid_act,
        "SCW": SCW or MCW,
        "key": (D, H, M, U, DP, K, NC, MCW, QCW, NRS, PAD, hid_act,
                SCW or MCW, tuple(offs.tolist())),
    }

    in_maps, m_idx = _prepare(inputs, cfg)
    results = _run(cfg, in_maps, trace=bool(os.environ.get("KERNEL_TRACE")),
                   sim=sim)

    deltaT = np.concatenate(
        [np.asarray(results[c]["out"], np.float32) for c in range(NC)], axis=0
    )  # [D, M]
    delta_md = deltaT.T  # [M, D]
    out = np.zeros((L, D), np.float32)
    if len(np.unique(m_idx)) == len(m_idx):
        out[m_idx] = delta_md
    else:
        np.add.at(out, m_idx, delta_md)
    return out[None]


def kernel(**inputs):
    return kernel_impl(
        inputs, D=4096, K=8, L=2048, M=1024, U=1024, DP=512, H=2048,
        NC=NCORES, MCW=512, QCW=512, NRS=2, SCW=256,
    )


# revision 63
# speedup vs baseline: 1.2124x; 1.2124x over previous
"""Trainium2 Bass kernel for nn_AMIPRouterInference (gnn_message_passing).

Strategy (8 NeuronCores, expert-parallel):
  - Each core owns one of the K=8 experts (weights read from HBM exactly once
    chip-wide).  The router / q / k weights are replicated (tiny).
  - The first MLP layer is deduplicated: instead of computing
    gelu(cond @ W1) per (mask, anchor) pair (10x redundant), we compute
    A1T once per anchor row and M1T once per mask row, then combine shifted
    planes.  All tensors live in a transposed [feature-partition,
    position-free] layout so the (anchor - mask) index offsets become
    free-axis shifts.
  - Pair combine weights (segment softmax * router gate) are computed as
    partition-0 rows via ones-vector column-sum matmuls, then broadcast to
    128 partitions with K=1 matmuls and folded into the plane accumulation,
    which shrinks the second MLP matmul by 10x as well.
  - Each core produces a full [D, M] partial delta (its expert, all masks);
    a ReduceScatter over the 8 cores sums the experts and leaves each core
    with a [D/8, M] slice, DMA'd out.  The host reassembles / transposes and
    scatters rows into the [1, L, D] output.

The pair tables (which (mask, anchor) pairs exist) are integer-only host
work derived from the runtime index inputs; they parameterize the compiled
graph (offset planes + validity masks).
"""

import os
import numpy as np

NCORES = 8

_GRAPH_CACHE = {}
LAST_RESULT = None  # BassKernelResults of the most recent device run


# ----------------------------------------------------------------------------
# Host-side pair-table construction (mirrors reference semantics exactly)
# ----------------------------------------------------------------------------

def build_tables(m_idx, u_idx, r, pmax):
    M = len(m_idx)
    dists = np.abs(m_idx[:, None].astype(np.int64) - u_idx[None, :].astype(np.int64))
    adj = (dists > 0) & (dists <= r)
    pair_m, pair_u = np.nonzero(adj)  # row-major == jnp.nonzero order
    pair_m = pair_m[:pmax]
    pair_u = pair_u[:pmax]
    offs = np.unique(pair_u - pair_m).astype(np.int64)
    J = len(offs)
    valid = np.zeros((J, M), dtype=np.float32)
    for j, d in enumerate(offs):
        valid[j, pair_m[(pair_u - pair_m) == d]] = 1.0
    return offs, valid


# ----------------------------------------------------------------------------
# Graph builder (SPMD: all cores run this graph with different input data)
# ----------------------------------------------------------------------------

def build_graph(cfg):
    import contextlib
    import concourse.mybir as mybir
    import concourse.tile as tile
    from concourse import bacc

    D, H, M, U, DP, K = cfg["D"], cfg["H"], cfg["M"], cfg["U"], cfg["DP"], cfg["K"]
    NC = cfg["NC"]
    offs = cfg["offs"]
    J = len(offs)
    PAD = cfg["PAD"]
    MCW = cfg["MCW"]            # compute chunk width along M
    NMC = M // MCW
    QCW = cfg["QCW"]            # qk/score-phase chunk width
    NQC = M // QCW
    DB, HB, DPB = D // 128, H // 128, DP // 128
    HGS = min(4, HB)            # h-blocks per A/M-phase psum group
    DGS = min(4, DB)            # d-blocks per W2-phase psum group
    RSD = D // NC               # rows of final output per core
    NRS = cfg["NRS"]            # number of reduce-scatter column groups
    RSW = M // NRS
    assert M % MCW == 0 and M % QCW == 0 and M % NRS == 0 and (RSW % MCW == 0)

    bf16 = mybir.dt.bfloat16
    f32 = mybir.dt.float32
    AF = mybir.ActivationFunctionType
    hid_af = getattr(AF, cfg.get("hid_act", "Gelu"))

    nc = bacc.Bacc(None, target_bir_lowering=False, debug=False)

    # ---------------- DRAM parameters ----------------
    hmT = nc.declare_dram_parameter("hmT", [D, M], bf16, isOutput=False)
    huT = nc.declare_dram_parameter("huT", [D, U], bf16, isOutput=False)
    w1a = nc.declare_dram_parameter("w1a", [D, H], bf16, isOutput=False)
    w1b = nc.declare_dram_parameter("w1b", [D, H], bf16, isOutput=False)
    w2 = nc.declare_dram_parameter("w2", [H, D], bf16, isOutput=False)
    # q/k are dp-sharded across cores: each core receives only its 128-wide
    # slice of Wq/Wk; raw scores are summed with a small AllReduce.
    wq = nc.declare_dram_parameter("wq", [D, 128], bf16, isOutput=False)
    wk = nc.declare_dram_parameter("wk", [D, 128], bf16, isOutput=False)
    wr = nc.declare_dram_parameter("wr", [D, K], bf16, isOutput=False)
    b1c = nc.declare_dram_parameter("b1c", [128, HB], f32, isOutput=False)
    b2r = nc.declare_dram_parameter("b2r", [1, D], bf16, isOutput=False)
    bqc = nc.declare_dram_parameter("bqc", [128, 1], f32, isOutput=False)
    bkc = nc.declare_dram_parameter("bkc", [128, 1], f32, isOutput=False)
    brc = nc.declare_dram_parameter("brc", [128, 1], f32, isOutput=False)
    esel = nc.declare_dram_parameter("esel", [K, 1], f32, isOutput=False)
    vmask = nc.declare_dram_parameter("vmask", [J, M], bf16, isOutput=False)
    outp = nc.declare_dram_parameter("out", [RSD, M], bf16, isOutput=True)

    with tile.TileContext(nc) as tc, contextlib.ExitStack() as ctx:
        sb = ctx.enter_context(tc.tile_pool(name="sb", bufs=1))
        ps = ctx.enter_context(tc.tile_pool(name="ps", bufs=1, space="PSUM"))
        dram = ctx.enter_context(tc.tile_pool(name="dram", bufs=1, space="DRAM"))

        def psum_mm(name):
            return ps.tile([128, 512], f32, tag="mm", bufs=6, name=name)

        def psum_row(name):
            return ps.tile([8, 512], f32, tag="row", bufs=2, name=name)

        # ---------------- persistent SBUF tensors ----------------
        ones = sb.tile([128, 128], bf16, name="ones")
        nc.vector.memset(ones[:, :], 1.0)
        ones32 = sb.tile([128, 1], f32, name="ones32")
        nc.vector.memset(ones32[:, :], 1.0)

        b1_sb = sb.tile([128, HB], f32, name="b1_sb")
        nc.sync.dma_start(b1_sb[:, :], b1c[:, :])
        bq_sb = sb.tile([128, 1], f32, name="bq_sb")
        nc.sync.dma_start(bq_sb[:, :], bqc[:, :])
        bk_sb = sb.tile([128, 1], f32, name="bk_sb")
        nc.sync.dma_start(bk_sb[:, :], bkc[:, :])
        br_sb = sb.tile([128, 1], f32, name="br_sb")
        nc.sync.dma_start(br_sb[:, :], brc[:, :])
        esel_sb = sb.tile([K, 1], f32, name="esel_sb")
        nc.sync.dma_start(esel_sb[:, :], esel[:, :])
        wr_sb = sb.tile([128, DB, K], bf16, name="wr_sb")
        nc.sync.dma_start(
            wr_sb[:, :, :], wr.ap().rearrange("(o p) k -> p o k", p=128)
        )
        b2_sb = sb.tile([1, D], bf16, name="b2_sb")
        nc.sync.dma_start(b2_sb[:, :], b2r[:, :])

        # steady-phase streaming tiles live in the persistent pool so their
        # addresses never overlap the phase-1/2 pool (overlap deps would
        # serialize the first A-phase against the score computation)
        def hmc_tile(name):
            return sb.tile([128, D // 128, cfg["MCW"]], bf16, tag="hmc",
                           bufs=2, name=name)

        def w1t_tile(name):
            return sb.tile([128, 4, min(4, H // 128) * 128], bf16, tag="w1t",
                           bufs=2, name=name)

        def w2t_tile(name):
            return sb.tile([128, 4, min(4, D // 128) * 128], bf16, tag="w2t",
                           bufs=2, name=name)

        A1T = sb.tile([128, HB, U + 2 * PAD], bf16, name="A1T")
        for hb in range(HB):
            nc.vector.memset(A1T[:, hb, 0:PAD], 0.0)
            nc.vector.memset(A1T[:, hb, PAD + U: U + 2 * PAD], 0.0)

        # combine-weight rows live in DRAM between phase 2 and the hid phases
        cw_dram = dram.tile([J, M], bf16, name="cw_dram")
        cwsum_bf = sb.tile([1, M], bf16, name="cwsum_bf")

        # ================= phases 1+2 (scoped pool) =================
        # raw-score AllReduce buffers (each core computes a 128-wide dp slice)
        sraw_b = dram.tile([J, M], f32, name="sraw_b")
        sred_b = dram.tile(
            [J, M], f32, name="sred_b",
            addr_space="Shared" if NC > 4 else "Local",
        )

        with tc.tile_pool(name="p12", bufs=1) as p12:
            kT_sb = p12.tile([128, U + 2 * PAD], bf16, name="kT_sb")
            nc.vector.memset(kT_sb[:, 0:PAD], 0.0)
            nc.vector.memset(kT_sb[:, PAD + U: U + 2 * PAD], 0.0)
            qT_sb = p12.tile([128, M], bf16, name="qT_sb")
            eg_sb = p12.tile([K, M], f32, name="eg_sb")

            # ---- phase 1: q / k / gate matmuls (q/k only a 128-dp slice) ----
            DBB = min(4, DB)    # d-blocks fetched per DMA
            for ch in range(NQC):
                csl = slice(ch * QCW, (ch + 1) * QCW)
                # q + gate sweep (rhs: hmT tiles streamed, batched fetches)
                psq = psum_mm("psq")
                psg = psum_row("psg")
                for dbb in range(0, DB, DBB):
                    hm_t = p12.tile([128, DBB, QCW], bf16, tag="ht", bufs=2,
                                    name="hm_t")
                    nc.sync.dma_start(
                        hm_t[:, :, :],
                        hmT[dbb * 128:(dbb + DBB) * 128, csl].rearrange(
                            "(o p) m -> p o m", p=128),
                    )
                    wq_t = p12.tile([128, DBB, 128], bf16, tag="wt", bufs=2,
                                    name="wq_t")
                    nc.sync.dma_start(
                        wq_t[:, :, :],
                        wq[dbb * 128:(dbb + DBB) * 128, :].rearrange(
                            "(o p) m -> p o m", p=128),
                    )
                    for i in range(DBB):
                        db = dbb + i
                        st, sp = db == 0, db == DB - 1
                        nc.tensor.matmul(
                            psq[:, :QCW], wq_t[:, i, :], hm_t[:, i, :],
                            start=st, stop=sp,
                        )
                        nc.tensor.matmul(
                            psg[:K, :QCW], wr_sb[:, db, :], hm_t[:, i, :],
                            start=st, stop=sp,
                        )
                nc.vector.tensor_scalar_add(
                    qT_sb[:, csl], psq[:, :QCW], bq_sb[:, 0:1],
                )
                nc.scalar.activation(
                    eg_sb[:, csl], psg[:K, :QCW], AF.Exp,
                    bias=br_sb[0:K, 0:1], scale=1.0,
                )
                # k sweep
                psk = psum_mm("psk")
                for dbb in range(0, DB, DBB):
                    hu_t = p12.tile([128, DBB, QCW], bf16, tag="ht", bufs=2,
                                    name="hu_t")
                    nc.sync.dma_start(
                        hu_t[:, :, :],
                        huT[dbb * 128:(dbb + DBB) * 128, csl].rearrange(
                            "(o p) m -> p o m", p=128),
                    )
                    wk_t = p12.tile([128, DBB, 128], bf16, tag="wt", bufs=2,
                                    name="wk_t")
                    nc.sync.dma_start(
                        wk_t[:, :, :],
                        wk[dbb * 128:(dbb + DBB) * 128, :].rearrange(
                            "(o p) m -> p o m", p=128),
                    )
                    for i in range(DBB):
                        db = dbb + i
                        nc.tensor.matmul(
                            psk[:, :QCW], wk_t[:, i, :], hu_t[:, i, :],
                            start=db == 0, stop=db == DB - 1,
                        )
                nc.vector.tensor_scalar_add(
                    kT_sb[:, PAD + ch * QCW: PAD + (ch + 1) * QCW],
                    psk[:, :QCW], bk_sb[:, 0:1],
                )

            # ---- raw scores (this core's dp-slice), then AllReduce ----
            for ch in range(NQC):
                for j in range(J):
                    off = int(offs[j])
                    pss = psum_row(f"pss{j}")
                    prod = p12.tile([128, QCW], bf16, tag="prod", bufs=2,
                                    name="prod")
                    nc.vector.tensor_mul(
                        out=prod[:, :],
                        in0=qT_sb[:, ch * QCW:(ch + 1) * QCW],
                        in1=kT_sb[:, PAD + off + ch * QCW:
                                  PAD + off + (ch + 1) * QCW],
                    )
                    nc.tensor.matmul(
                        pss[0:1, :QCW], ones[:, 0:1], prod[:, :],
                        start=True, stop=True,
                    )
                    s_t = p12.tile([1, QCW], f32, tag="st", bufs=2, name="s_t")
                    nc.vector.tensor_copy(s_t[:, :], pss[0:1, :QCW])
                    nc.sync.dma_start(
                        sraw_b[j:j + 1, ch * QCW:(ch + 1) * QCW], s_t[:, :]
                    )
            nc.gpsimd.collective_compute(
                "AllReduce",
                mybir.AluOpType.add,
                ins=[sraw_b.opt()],
                outs=[sred_b.opt()],
                replica_groups=[list(range(NC))],
            )

            # ---- phase 2: softmax / gate / combine rows ----
            # every dp-slice is contributed NC/ceil(DP/128) times
            ncopies = max(1, NC // max(1, DP // 128))
            inv_sqrt_dp = 1.0 / (float(np.sqrt(DP)) * ncopies)
            P2W = min(QCW, 256)
            for ch in range(M // P2W):
                csl2 = slice(ch * P2W, (ch + 1) * P2W)
                ej = p12.tile([1, J * P2W], f32, tag="ej", bufs=1, name="ej")
                vm_ch = p12.tile([1, J * P2W], bf16, tag="vm", bufs=1,
                                 name="vm_ch")
                sr_ch = p12.tile([1, J * P2W], f32, tag="sr", bufs=1,
                                 name="sr_ch")
                for j in range(J):
                    nc.sync.dma_start(
                        vm_ch[0:1, j * P2W:(j + 1) * P2W], vmask[j:j + 1, csl2]
                    )
                    nc.sync.dma_start(
                        sr_ch[0:1, j * P2W:(j + 1) * P2W], sred_b[j:j + 1, csl2]
                    )
                rowt = p12.tile([1, 6 * P2W], f32, tag="rw", bufs=1, name="rowt")

                def rw(i):
                    return rowt[0:1, i * P2W:(i + 1) * P2W]

                for j in range(J):
                    ejr = ej[0:1, j * P2W:(j + 1) * P2W]
                    nc.scalar.activation(
                        ejr, sr_ch[0:1, j * P2W:(j + 1) * P2W], AF.Exp,
                        bias=0.0, scale=inv_sqrt_dp,
                    )
                    nc.vector.tensor_mul(
                        out=ejr, in0=ejr,
                        in1=vm_ch[0:1, j * P2W:(j + 1) * P2W],
                    )
                # ssum / srecip
                nc.vector.tensor_add(
                    out=rw(0), in0=ej[0:1, 0:P2W], in1=ej[0:1, P2W:2 * P2W]
                )
                for j in range(2, J):
                    nc.vector.tensor_add(
                        out=rw(0), in0=rw(0),
                        in1=ej[0:1, j * P2W:(j + 1) * P2W],
                    )
                nc.vector.tensor_scalar_max(rw(1), rw(0), 1e-8)
                nc.vector.reciprocal(rw(1), rw(1))
                # gate: gsum over K partitions, recip, selected expert
                psr = psum_row("psgs")
                nc.tensor.matmul(
                    psr[0:1, :P2W], ones32[0:K, 0:1], eg_sb[:, csl2],
                    start=True, stop=True,
                )
                nc.vector.tensor_copy(rw(2), psr[0:1, :P2W])
                nc.vector.reciprocal(rw(2), rw(2))
                pse = psum_row("pse")
                nc.tensor.matmul(
                    pse[0:1, :P2W], esel_sb[:, 0:1], eg_sb[:, csl2],
                    start=True, stop=True,
                )
                nc.vector.tensor_copy(rw(3), pse[0:1, :P2W])
                # ge = sel * grecip ; rg = srecip * ge
                nc.vector.tensor_mul(out=rw(3), in0=rw(3), in1=rw(2))
                nc.vector.tensor_mul(out=rw(4), in0=rw(1), in1=rw(3))
                # cw_j = ej * rg ; cwsum = ssum * rg
                cwrow = p12.tile([1, J * P2W], bf16, tag="cwr", bufs=1,
                                 name="cwrow")
                for j in range(J):
                    nc.vector.tensor_mul(
                        out=cwrow[0:1, j * P2W:(j + 1) * P2W],
                        in0=ej[0:1, j * P2W:(j + 1) * P2W], in1=rw(4),
                    )
                for j in range(J):
                    nc.sync.dma_start(
                        cw_dram[j:j + 1, csl2],
                        cwrow[0:1, j * P2W:(j + 1) * P2W],
                    )
                nc.vector.tensor_mul(out=rw(5), in0=rw(0), in1=rw(4))
                nc.vector.tensor_copy(cwsum_bf[0:1, csl2], rw(5))

        # ================= steady phases (scoped pools) =================
        SCW = cfg.get("SCW", MCW)       # hid-phase sub-chunk width
        NSC = MCW // SCW
        with tc.tile_pool(name="hresp", bufs=1) as hres, \
             tc.tile_pool(name="streamp", bufs=1) as stream, \
             tc.tile_pool(name="tmpp", bufs=1) as tmp:

            def msl(mc):
                return slice(mc * MCW, (mc + 1) * MCW)

            HGRP = [list(range(g, min(g + HGS, HB))) for g in range(0, HB, HGS)]
            DGRP = [list(range(g, min(g + DGS, DB))) for g in range(0, DB, DGS)]

            bounce = [
                dram.tile([D, RSW], bf16, name=f"bounce{g}") for g in range(NRS)
            ]
            rsout = [
                dram.tile([RSD, RSW], bf16, name=f"rsout{g}") for g in range(NRS)
            ]

            def a_or_m_phase(mc, which, m1b_t=None):
                src, wsrc = (huT, w1a) if which == "a" else (hmT, w1b)
                h_mc = hmc_tile(f"h_{which}{mc}")
                nc.sync.dma_start(
                    h_mc[:, :, :],
                    src.ap().rearrange("(o p) m -> p o m", p=128)[:, :, msl(mc)],
                )
                DBB = min(4, DB)
                for grp in HGRP:
                    gw = len(grp) * 128
                    psa = [psum_mm(f"ps{which}{i}") for i in range(len(grp))]
                    for dbb in range(0, DB, DBB):
                        w_t = w1t_tile("w1_t")
                        nc.sync.dma_start(
                            w_t[:, :DBB, :gw],
                            wsrc[dbb * 128:(dbb + DBB) * 128,
                                 grp[0] * 128: grp[0] * 128 + gw].rearrange(
                                     "(o p) h -> p o h", p=128),
                        )
                        for i in range(DBB):
                            db = dbb + i
                            for gi, hb in enumerate(grp):
                                nc.tensor.matmul(
                                    psa[gi][:, :MCW],
                                    w_t[:, i, gi * 128:(gi + 1) * 128],
                                    h_mc[:, db, :],
                                    start=db == 0, stop=db == DB - 1,
                                )
                    # psum -> SBUF casts on ScalarE (DVE is the busy engine)
                    for gi, hb in enumerate(grp):
                        if which == "a":
                            nc.scalar.activation(
                                A1T[:, hb, PAD + mc * MCW: PAD + (mc + 1) * MCW],
                                psa[gi][:, :MCW], AF.Identity,
                                bias=0.0, scale=1.0,
                            )
                        else:
                            for s in range(NSC):
                                nc.scalar.activation(
                                    m1b_t[s][:, hb, :],
                                    psa[gi][:, s * SCW:(s + 1) * SCW],
                                    AF.Identity,
                                    bias=b1_sb[:, hb:hb + 1], scale=1.0,
                                )

            def cwb_build(mc):
                # broadcast combine-weight rows to 128 partitions, early so
                # the K=1 matmuls slot into the A-phase and hid never waits
                cwsl = tmp.tile([1, J * MCW], bf16, tag="cwsl", bufs=1,
                                name="cwsl")
                for j in range(J):
                    nc.sync.dma_start(
                        cwsl[0:1, j * MCW:(j + 1) * MCW],
                        cw_dram[j:j + 1, msl(mc)],
                    )
                cwbs = []
                for s in range(NSC):
                    cwb = tmp.tile([128, J, SCW], bf16, tag="cwb", bufs=3,
                                   name=f"cwb{mc}_{s}")
                    for j in range(J):
                        psb = psum_mm(f"psb{j}")
                        nc.tensor.matmul(
                            psb[:, :SCW], ones[0:1, :],
                            cwsl[0:1, j * MCW + s * SCW: j * MCW + (s + 1) * SCW],
                            start=True, stop=True,
                        )
                        nc.vector.tensor_copy(cwb[:, j, :], psb[:, :SCW])
                    cwbs.append(cwb)
                return cwbs

            def hid_sub(mc, s, m1b_t, hid_t, cwb):
                c0 = mc * MCW + s * SCW
                for hb in range(HB):
                    for j in range(J):
                        off = int(offs[j])
                        x_t = tmp.tile([128, SCW], bf16, tag="xt", bufs=2,
                                       name="x_t")
                        nc.vector.tensor_add(
                            out=x_t[:, :],
                            in0=A1T[:, hb, PAD + off + c0: PAD + off + c0 + SCW],
                            in1=m1b_t[:, hb, :],
                        )
                        g_t = tmp.tile([128, SCW], bf16, tag="gt", bufs=2,
                                       name="g_t")
                        nc.scalar.activation(
                            g_t[:, :], x_t[:, :], hid_af, bias=0.0, scale=1.0,
                        )
                        dst = hid_t[:, hb, s * SCW:(s + 1) * SCW]
                        if j == 0:
                            nc.vector.tensor_mul(
                                out=dst, in0=g_t[:, :], in1=cwb[:, j, :],
                            )
                        else:
                            nc.vector.tensor_mul(
                                out=g_t[:, :], in0=g_t[:, :], in1=cwb[:, j, :]
                            )
                            nc.vector.tensor_add(out=dst, in0=dst, in1=g_t[:, :])

            def w2_mc(mc, hid_t):
                c0 = mc * MCW
                g = c0 // RSW
                col0 = c0 - g * RSW
                HBB = min(4, HB)
                for grp in DGRP:
                    gw = len(grp) * 128
                    psd = [psum_mm(f"psd{i}") for i in range(len(grp))]
                    for hbb in range(0, HB, HBB):
                        w2_t = w2t_tile("w2_t")
                        nc.sync.dma_start(
                            w2_t[:, :HBB, :gw],
                            w2[hbb * 128:(hbb + HBB) * 128,
                               grp[0] * 128: grp[0] * 128 + gw].rearrange(
                                   "(o p) d -> p o d", p=128),
                        )
                        for i in range(HBB):
                            hb = hbb + i
                            for gi, db in enumerate(grp):
                                nc.tensor.matmul(
                                    psd[gi][:, :MCW],
                                    w2_t[:, i, gi * 128:(gi + 1) * 128],
                                    hid_t[:, hb, :],
                                    start=hb == 0, stop=False,
                                )
                    for gi, db in enumerate(grp):
                        nc.tensor.matmul(
                            psd[gi][:, :MCW],
                            b2_sb[0:1, db * 128:(db + 1) * 128],
                            cwsum_bf[0:1, c0:c0 + MCW],
                            start=False, stop=True,
                        )
                        d_t = tmp.tile([128, MCW], bf16, tag="dt", bufs=1,
                                       name="d_t")
                        # psum -> SBUF on ScalarE: keeps DVE free for hid work
                        nc.scalar.activation(
                            d_t[:, :], psd[gi][:, :MCW], AF.Identity,
                            bias=0.0, scale=1.0,
                        )
                        nc.sync.dma_start(
                            bounce[g][db * 128:(db + 1) * 128,
                                      col0:col0 + MCW],
                            d_t[:, :],
                        )

            def rs_maybe(mc):
                if (mc + 1) % max(1, RSW // MCW) == 0:
                    g = (mc * MCW) // RSW
                    nc.gpsimd.collective_compute(
                        "ReduceScatter",
                        mybir.AluOpType.add,
                        ins=[bounce[g].opt()],
                        outs=[rsout[g].opt()],
                        replica_groups=[list(range(NC))],
                    )
                    nc.sync.dma_start(
                        outp[:, g * RSW:(g + 1) * RSW], rsout[g][:, :]
                    )

            m1b_store, hid_store, cwb_store = {}, {}, {}

            for mc in range(NMC):
                a_or_m_phase(mc, "a")
                cwb_store[mc] = cwb_build(mc)
                m1b_store[mc] = [
                    tmp.tile([128, HB, SCW], bf16, tag="m1b", bufs=3,
                             name=f"m1b_{mc}_{s}")
                    for s in range(NSC)
                ]
                a_or_m_phase(mc, "m", m1b_store[mc])
                hid_store[mc] = tmp.tile([128, HB, MCW], bf16, tag="hidct",
                                         bufs=2, name=f"hid_{mc}")
                if mc >= 1:
                    # previous chunk's last sub-chunk needed this A's halo
                    pm, ls = mc - 1, NSC - 1
                    hid_sub(pm, ls, m1b_store[pm][ls], hid_store[pm],
                            cwb_store[pm][ls])
                    w2_mc(pm, hid_store[pm])
                    rs_maybe(pm)
                    del m1b_store[pm], cwb_store[pm], hid_store[pm]
                for s in range(NSC - 1):
                    hid_sub(mc, s, m1b_store[mc][s], hid_store[mc],
                            cwb_store[mc][s])
            mc, ls = NMC - 1, NSC - 1
            hid_sub(mc, ls, m1b_store[mc][ls], hid_store[mc], cwb_store[mc][ls])
            w2_mc(mc, hid_store[mc])
            rs_maybe(mc)

    nc.finalize()
    return nc


# ----------------------------------------------------------------------------
# Host wrapper
# ----------------------------------------------------------------------------

def _prepare(inputs, cfg):
    import ml_dtypes
    BF16 = ml_dtypes.bfloat16
    D, H, M, U, DP, K = cfg["D"], cfg["H"], cfg["M"], cfg["U"], cfg["DP"], cfg["K"]
    HB, DPB = H // 128, DP // 128
    offs, valid = cfg["offs"], cfg["valid"]
    J = len(offs)
    QCW = cfg["QCW"]
    NQC = M // QCW

    h = np.asarray(inputs["h_L"], dtype=np.float32)[0]
    m_idx = np.asarray(inputs["mask_indices"]).astype(np.int64)
    u_idx = np.asarray(inputs["unmasked_indices"]).astype(np.int64)

    hmT = np.ascontiguousarray(h[m_idx].astype(BF16).T)
    huT = np.ascontiguousarray(h[u_idx].astype(BF16).T)
    wq = np.asarray(inputs["Wq"], np.float32).astype(BF16)
    wk = np.asarray(inputs["Wk"], np.float32).astype(BF16)
    wr = np.asarray(inputs["Wr"], np.float32).astype(BF16)
    bq = np.asarray(inputs["bq"], np.float32)
    bk = np.asarray(inputs["bk"], np.float32)
    brc = np.zeros((128, 1), np.float32)
    brc[:K, 0] = np.asarray(inputs["br"], np.float32)
    vm = np.ascontiguousarray(valid).astype(BF16)  # [J, M]

    W1 = np.asarray(inputs["W1"], np.float32)
    W2 = np.asarray(inputs["W2"], np.float32)
    b1 = np.asarray(inputs["b1"], np.float32)
    b2 = np.asarray(inputs["b2"], np.float32)

    DPBT = max(1, DP // 128)
    in_maps = []
    for c in range(cfg["NC"]):
        e = c % K
        dpb = c % DPBT
        dsl = slice(dpb * 128, (dpb + 1) * 128)
        sel = np.zeros((K, 1), np.float32)
        sel[e, 0] = 1.0
        in_maps.append({
            "hmT": hmT, "huT": huT,
            "w1a": np.ascontiguousarray(W1[e][:D]).astype(BF16),
            "w1b": np.ascontiguousarray(W1[e][D:]).astype(BF16),
            "w2": W2[e].astype(BF16),
            "wq": np.ascontiguousarray(wq[:, dsl]),
            "wk": np.ascontiguousarray(wk[:, dsl]),
            "wr": wr,
            "b1c": np.ascontiguousarray(b1[e].reshape(HB, 128).T),
            "b2r": b2[e].reshape(1, D).astype(BF16),
            "bqc": np.ascontiguousarray(bq[dsl].reshape(128, 1)),
            "bkc": np.ascontiguousarray(bk[dsl].reshape(128, 1)),
            "brc": brc,
            "esel": sel, "vmask": vm,
        })
    return in_maps, m_idx


def _run(cfg, in_maps, trace=False, sim=False):
    global LAST_RESULT
    key = cfg["key"]
    if key not in _GRAPH_CACHE:
        _GRAPH_CACHE[key] = build_graph(cfg)
    nc = _GRAPH_CACHE[key]
    if sim:
        from concourse import bass_interp
        s = bass_interp.MultiCoreSim(nc, cfg["NC"])
        for c in range(cfg["NC"]):
            for k, v in in_maps[c].items():
                s.cores[c].tensor(k)[:] = v
        s.simulate(check_with_hw=False)
        return [{"out": np.asarray(s.cores[c].mem_tensor("out"))}
                for c in range(cfg["NC"])]
    from concourse import bass_utils
    kw = {}
    if trace and os.environ.get("KERNEL_TRACE_DIR"):
        kw["tmpdir"] = os.environ["KERNEL_TRACE_DIR"]
    res = bass_utils.run_bass_kernel_spmd(
        nc, in_maps, core_ids=list(range(cfg["NC"])), trace=trace, **kw,
    )
    LAST_RESULT = res
    return res.results


def kernel_impl(inputs, D, K, L, M, U, DP, H, NC, MCW, QCW, NRS, sim=False,
                hid_act="Gelu", SCW=None):
    PMAX = M * 10

    m_idx = np.asarray(inputs["mask_indices"]).astype(np.int64)
    u_idx = np.asarray(inputs["unmasked_indices"]).astype(np.int64)
    r = int(np.asarray(inputs["range_r"]))

    offs, valid = build_tables(m_idx, u_idx, r, PMAX)
    J = len(offs)
    if J == 0:
        return np.zeros((1, L, D), np.float32)
    PAD = int(max(8, np.max(np.abs(offs))))
    PAD = (PAD + 7) // 8 * 8

    cfg = {
        "D": D, "H": H, "M": M, "U": U, "DP": DP, "K": K, "NC": NC,
        "offs": offs, "valid": valid, "PAD": PAD,
        "MCW": MCW, "QCW": QCW, "NRS": NRS, "hid_act": hid_act,
        "SCW": SCW or MCW,
        "key": (D, H, M, U, DP, K, NC, MCW, QCW, NRS, PAD, hid_act,
                SCW or MCW, tuple(offs.tolist())),
    }

    in_maps, m_idx = _prepare(inputs, cfg)
    results = _run(cfg, in_maps, trace=bool(os.environ.get("KERNEL_TRACE")),
                   sim=sim)

    deltaT = np.concatenate(
        [np.asarray(results[c]["out"], np.float32) for c in range(NC)], axis=0
    )  # [D, M]
    delta_md = deltaT.T  # [M, D]
    out = np.zeros((L, D), np.float32)
    if len(np.unique(m_idx)) == len(m_idx):
        out[m_idx] = delta_md
    else:
        np.add.at(out, m_idx, delta_md)
    return out[None]


def kernel(**inputs):
    return kernel_impl(
        inputs, D=4096, K=8, L=2048, M=1024, U=1024, DP=512, H=2048,
        NC=NCORES, MCW=512, QCW=512, NRS=2, SCW=256,
    )


# revision 64
# speedup vs baseline: 1.2359x; 1.0194x over previous
"""Trainium2 Bass kernel for nn_AMIPRouterInference (gnn_message_passing).

Strategy (8 NeuronCores, expert-parallel):
  - Each core owns one of the K=8 experts (weights read from HBM exactly once
    chip-wide).  The router / q / k weights are replicated (tiny).
  - The first MLP layer is deduplicated: instead of computing
    gelu(cond @ W1) per (mask, anchor) pair (10x redundant), we compute
    A1T once per anchor row and M1T once per mask row, then combine shifted
    planes.  All tensors live in a transposed [feature-partition,
    position-free] layout so the (anchor - mask) index offsets become
    free-axis shifts.
  - Pair combine weights (segment softmax * router gate) are computed as
    partition-0 rows via ones-vector column-sum matmuls, then broadcast to
    128 partitions with K=1 matmuls and folded into the plane accumulation,
    which shrinks the second MLP matmul by 10x as well.
  - Each core produces a full [D, M] partial delta (its expert, all masks);
    a ReduceScatter over the 8 cores sums the experts and leaves each core
    with a [D/8, M] slice, DMA'd out.  The host reassembles / transposes and
    scatters rows into the [1, L, D] output.

The pair tables (which (mask, anchor) pairs exist) are integer-only host
work derived from the runtime index inputs; they parameterize the compiled
graph (offset planes + validity masks).
"""

import os
import numpy as np

NCORES = 8

_GRAPH_CACHE = {}
LAST_RESULT = None  # BassKernelResults of the most recent device run


# ----------------------------------------------------------------------------
# Host-side pair-table construction (mirrors reference semantics exactly)
# ----------------------------------------------------------------------------

def build_tables(m_idx, u_idx, r, pmax):
    M = len(m_idx)
    dists = np.abs(m_idx[:, None].astype(np.int64) - u_idx[None, :].astype(np.int64))
    adj = (dists > 0) & (dists <= r)
    pair_m, pair_u = np.nonzero(adj)  # row-major == jnp.nonzero order
    pair_m = pair_m[:pmax]
    pair_u = pair_u[:pmax]
    offs = np.unique(pair_u - pair_m).astype(np.int64)
    J = len(offs)
    valid = np.zeros((J, M), dtype=np.float32)
    for j, d in enumerate(offs):
        valid[j, pair_m[(pair_u - pair_m) == d]] = 1.0
    return offs, valid


# ----------------------------------------------------------------------------
# Graph builder (SPMD: all cores run this graph with different input data)
# ----------------------------------------------------------------------------

def build_graph(cfg):
    import contextlib
    import concourse.mybir as mybir
    import concourse.tile as tile
    from concourse import bacc

    D, H, M, U, DP, K = cfg["D"], cfg["H"], cfg["M"], cfg["U"], cfg["DP"], cfg["K"]
    NC = cfg["NC"]
    offs = cfg["offs"]
    J = len(offs)
    PAD = cfg["PAD"]
    MCW = cfg["MCW"]            # compute chunk width along M
    NMC = M // MCW
    QCW = cfg["QCW"]            # qk/score-phase chunk width
    NQC = M // QCW
    DB, HB, DPB = D // 128, H // 128, DP // 128
    HGS = min(4, HB)            # h-blocks per A/M-phase psum group
    DGS = min(4, DB)            # d-blocks per W2-phase psum group
    RSD = D // NC               # rows of final output per core
    NRS = cfg["NRS"]            # number of reduce-scatter column groups
    RSW = M // NRS
    assert M % MCW == 0 and M % QCW == 0 and M % NRS == 0 and (RSW % MCW == 0)

    bf16 = mybir.dt.bfloat16
    f32 = mybir.dt.float32
    AF = mybir.ActivationFunctionType
    hid_af = getattr(AF, cfg.get("hid_act", "Gelu"))

    nc = bacc.Bacc(None, target_bir_lowering=False, debug=False)

    # ---------------- DRAM parameters ----------------
    hmT = nc.declare_dram_parameter("hmT", [D, M], bf16, isOutput=False)
    huT = nc.declare_dram_parameter("huT", [D, U], bf16, isOutput=False)
    w1a = nc.declare_dram_parameter("w1a", [D, H], bf16, isOutput=False)
    w1b = nc.declare_dram_parameter("w1b", [D, H], bf16, isOutput=False)
    w2 = nc.declare_dram_parameter("w2", [H, D], bf16, isOutput=False)
    # q/k are dp-sharded across cores: each core receives only its 128-wide
    # slice of Wq/Wk; raw scores are summed with a small AllReduce.
    wq = nc.declare_dram_parameter("wq", [D, 128], bf16, isOutput=False)
    wk = nc.declare_dram_parameter("wk", [D, 128], bf16, isOutput=False)
    wr = nc.declare_dram_parameter("wr", [D, K], bf16, isOutput=False)
    b1c = nc.declare_dram_parameter("b1c", [128, HB], f32, isOutput=False)
    b2r = nc.declare_dram_parameter("b2r", [1, D], bf16, isOutput=False)
    bqc = nc.declare_dram_parameter("bqc", [128, 1], f32, isOutput=False)
    bkc = nc.declare_dram_parameter("bkc", [128, 1], f32, isOutput=False)
    brc = nc.declare_dram_parameter("brc", [128, 1], f32, isOutput=False)
    esel = nc.declare_dram_parameter("esel", [K, 1], f32, isOutput=False)
    vmask = nc.declare_dram_parameter("vmask", [J, M], bf16, isOutput=False)
    outp = nc.declare_dram_parameter("out", [RSD, M], bf16, isOutput=True)

    with tile.TileContext(nc) as tc, contextlib.ExitStack() as ctx:
        sb = ctx.enter_context(tc.tile_pool(name="sb", bufs=1))
        ps = ctx.enter_context(tc.tile_pool(name="ps", bufs=1, space="PSUM"))
        dram = ctx.enter_context(tc.tile_pool(name="dram", bufs=1, space="DRAM"))

        def psum_mm(name):
            return ps.tile([128, 512], f32, tag="mm", bufs=6, name=name)

        def psum_row(name):
            return ps.tile([8, 512], f32, tag="row", bufs=2, name=name)

        # ---------------- persistent SBUF tensors ----------------
        ones = sb.tile([128, 128], bf16, name="ones")
        nc.vector.memset(ones[:, :], 1.0)
        ones32 = sb.tile([128, 1], f32, name="ones32")
        nc.vector.memset(ones32[:, :], 1.0)

        b1_sb = sb.tile([128, HB], f32, name="b1_sb")
        nc.sync.dma_start(b1_sb[:, :], b1c[:, :])
        bq_sb = sb.tile([128, 1], f32, name="bq_sb")
        nc.sync.dma_start(bq_sb[:, :], bqc[:, :])
        bk_sb = sb.tile([128, 1], f32, name="bk_sb")
        nc.sync.dma_start(bk_sb[:, :], bkc[:, :])
        br_sb = sb.tile([128, 1], f32, name="br_sb")
        nc.sync.dma_start(br_sb[:, :], brc[:, :])
        esel_sb = sb.tile([K, 1], f32, name="esel_sb")
        nc.sync.dma_start(esel_sb[:, :], esel[:, :])
        wr_sb = sb.tile([128, DB, K], bf16, name="wr_sb")
        nc.sync.dma_start(
            wr_sb[:, :, :], wr.ap().rearrange("(o p) k -> p o k", p=128)
        )
        b2_sb = sb.tile([1, D], bf16, name="b2_sb")
        nc.sync.dma_start(b2_sb[:, :], b2r[:, :])

        # steady-phase streaming tiles live in the persistent pool so their
        # addresses never overlap the phase-1/2 pool (overlap deps would
        # serialize the first A-phase against the score computation)
        def hmc_tile(name):
            return sb.tile([128, D // 128, cfg["MCW"]], bf16, tag="hmc",
                           bufs=2, name=name)

        def w1t_tile(name):
            return sb.tile([128, 4, min(4, H // 128) * 128], bf16, tag="w1t",
                           bufs=2, name=name)

        def w2t_tile(name):
            return sb.tile([128, 4, min(4, D // 128) * 128], bf16, tag="w2t",
                           bufs=2, name=name)

        A1T = sb.tile([128, HB, U + 2 * PAD], bf16, name="A1T")
        for hb in range(HB):
            nc.vector.memset(A1T[:, hb, 0:PAD], 0.0)
            nc.vector.memset(A1T[:, hb, PAD + U: U + 2 * PAD], 0.0)

        # combine-weight rows live in DRAM between phase 2 and the hid phases
        cw_dram = dram.tile([J, M], bf16, name="cw_dram")
        cwsum_bf = sb.tile([1, M], bf16, name="cwsum_bf")

        # ================= phases 1+2 (scoped pool) =================
        # raw-score AllReduce buffers (each core computes a 128-wide dp slice)
        sraw_b = dram.tile([J, M], f32, name="sraw_b")
        sred_b = dram.tile(
            [J, M], f32, name="sred_b",
            addr_space="Shared" if NC > 4 else "Local",
        )

        with tc.tile_pool(name="p12", bufs=1) as p12:
            kT_sb = p12.tile([128, U + 2 * PAD], bf16, name="kT_sb")
            nc.vector.memset(kT_sb[:, 0:PAD], 0.0)
            nc.vector.memset(kT_sb[:, PAD + U: U + 2 * PAD], 0.0)
            qT_sb = p12.tile([128, M], bf16, name="qT_sb")
            eg_sb = p12.tile([K, M], f32, name="eg_sb")

            # ---- phase 1: q / k / gate matmuls (q/k only a 128-dp slice) ----
            DBB = min(4, DB)    # d-blocks fetched per DMA
            for ch in range(NQC):
                csl = slice(ch * QCW, (ch + 1) * QCW)
                # q + gate sweep (rhs: hmT tiles streamed, batched fetches)
                psq = psum_mm("psq")
                psg = psum_row("psg")
                for dbb in range(0, DB, DBB):
                    hm_t = p12.tile([128, DBB, QCW], bf16, tag="ht", bufs=2,
                                    name="hm_t")
                    nc.sync.dma_start(
                        hm_t[:, :, :],
                        hmT[dbb * 128:(dbb + DBB) * 128, csl].rearrange(
                            "(o p) m -> p o m", p=128),
                    )
                    wq_t = p12.tile([128, DBB, 128], bf16, tag="wt", bufs=2,
                                    name="wq_t")
                    nc.sync.dma_start(
                        wq_t[:, :, :],
                        wq[dbb * 128:(dbb + DBB) * 128, :].rearrange(
                            "(o p) m -> p o m", p=128),
                    )
                    for i in range(DBB):
                        db = dbb + i
                        st, sp = db == 0, db == DB - 1
                        nc.tensor.matmul(
                            psq[:, :QCW], wq_t[:, i, :], hm_t[:, i, :],
                            start=st, stop=sp,
                        )
                        nc.tensor.matmul(
                            psg[:K, :QCW], wr_sb[:, db, :], hm_t[:, i, :],
                            start=st, stop=sp,
                        )
                nc.vector.tensor_scalar_add(
                    qT_sb[:, csl], psq[:, :QCW], bq_sb[:, 0:1],
                )
                nc.scalar.activation(
                    eg_sb[:, csl], psg[:K, :QCW], AF.Exp,
                    bias=br_sb[0:K, 0:1], scale=1.0,
                )
                # k sweep
                psk = psum_mm("psk")
                for dbb in range(0, DB, DBB):
                    hu_t = p12.tile([128, DBB, QCW], bf16, tag="ht", bufs=2,
                                    name="hu_t")
                    nc.sync.dma_start(
                        hu_t[:, :, :],
                        huT[dbb * 128:(dbb + DBB) * 128, csl].rearrange(
                            "(o p) m -> p o m", p=128),
                    )
                    wk_t = p12.tile([128, DBB, 128], bf16, tag="wt", bufs=2,
                                    name="wk_t")
                    nc.sync.dma_start(
                        wk_t[:, :, :],
                        wk[dbb * 128:(dbb + DBB) * 128, :].rearrange(
                            "(o p) m -> p o m", p=128),
                    )
                    for i in range(DBB):
                        db = dbb + i
                        nc.tensor.matmul(
                            psk[:, :QCW], wk_t[:, i, :], hu_t[:, i, :],
                            start=db == 0, stop=db == DB - 1,
                        )
                nc.vector.tensor_scalar_add(
                    kT_sb[:, PAD + ch * QCW: PAD + (ch + 1) * QCW],
                    psk[:, :QCW], bk_sb[:, 0:1],
                )

            # ---- raw scores (this core's dp-slice), then AllReduce ----
            for ch in range(NQC):
                for j in range(J):
                    off = int(offs[j])
                    pss = psum_row(f"pss{j}")
                    prod = p12.tile([128, QCW], bf16, tag="prod", bufs=2,
                                    name="prod")
                    nc.vector.tensor_mul(
                        out=prod[:, :],
                        in0=qT_sb[:, ch * QCW:(ch + 1) * QCW],
                        in1=kT_sb[:, PAD + off + ch * QCW:
                                  PAD + off + (ch + 1) * QCW],
                    )
                    nc.tensor.matmul(
                        pss[0:1, :QCW], ones[:, 0:1], prod[:, :],
                        start=True, stop=True,
                    )
                    s_t = p12.tile([1, QCW], f32, tag="st", bufs=2, name="s_t")
                    nc.vector.tensor_copy(s_t[:, :], pss[0:1, :QCW])
                    nc.gpsimd.dma_start(
                        sraw_b[j:j + 1, ch * QCW:(ch + 1) * QCW], s_t[:, :]
                    )
            nc.gpsimd.collective_compute(
                "AllReduce",
                mybir.AluOpType.add,
                ins=[sraw_b.opt()],
                outs=[sred_b.opt()],
                replica_groups=[list(range(NC))],
            )

            # ---- phase 2: softmax / gate / combine rows ----
            # every dp-slice is contributed NC/ceil(DP/128) times
            ncopies = max(1, NC // max(1, DP // 128))
            inv_sqrt_dp = 1.0 / (float(np.sqrt(DP)) * ncopies)
            P2W = min(QCW, 256)
            for ch in range(M // P2W):
                csl2 = slice(ch * P2W, (ch + 1) * P2W)
                ej = p12.tile([1, J * P2W], f32, tag="ej", bufs=1, name="ej")
                vm_ch = p12.tile([1, J * P2W], bf16, tag="vm", bufs=1,
                                 name="vm_ch")
                sr_ch = p12.tile([1, J * P2W], f32, tag="sr", bufs=1,
                                 name="sr_ch")
                for j in range(J):
                    nc.gpsimd.dma_start(
                        vm_ch[0:1, j * P2W:(j + 1) * P2W], vmask[j:j + 1, csl2]
                    )
                    nc.gpsimd.dma_start(
                        sr_ch[0:1, j * P2W:(j + 1) * P2W], sred_b[j:j + 1, csl2]
                    )
                rowt = p12.tile([1, 6 * P2W], f32, tag="rw", bufs=1, name="rowt")

                def rw(i):
                    return rowt[0:1, i * P2W:(i + 1) * P2W]

                for j in range(J):
                    ejr = ej[0:1, j * P2W:(j + 1) * P2W]
                    nc.scalar.activation(
                        ejr, sr_ch[0:1, j * P2W:(j + 1) * P2W], AF.Exp,
                        bias=0.0, scale=inv_sqrt_dp,
                    )
                    nc.vector.tensor_mul(
                        out=ejr, in0=ejr,
                        in1=vm_ch[0:1, j * P2W:(j + 1) * P2W],
                    )
                # ssum / srecip
                nc.vector.tensor_add(
                    out=rw(0), in0=ej[0:1, 0:P2W], in1=ej[0:1, P2W:2 * P2W]
                )
                for j in range(2, J):
                    nc.vector.tensor_add(
                        out=rw(0), in0=rw(0),
                        in1=ej[0:1, j * P2W:(j + 1) * P2W],
                    )
                nc.vector.tensor_scalar_max(rw(1), rw(0), 1e-8)
                nc.vector.reciprocal(rw(1), rw(1))
                # gate: gsum over K partitions, recip, selected expert
                psr = psum_row("psgs")
                nc.tensor.matmul(
                    psr[0:1, :P2W], ones32[0:K, 0:1], eg_sb[:, csl2],
                    start=True, stop=True,
                )
                nc.vector.tensor_copy(rw(2), psr[0:1, :P2W])
                nc.vector.reciprocal(rw(2), rw(2))
                pse = psum_row("pse")
                nc.tensor.matmul(
                    pse[0:1, :P2W], esel_sb[:, 0:1], eg_sb[:, csl2],
                    start=True, stop=True,
                )
                nc.vector.tensor_copy(rw(3), pse[0:1, :P2W])
                # ge = sel * grecip ; rg = srecip * ge
                nc.vector.tensor_mul(out=rw(3), in0=rw(3), in1=rw(2))
                nc.vector.tensor_mul(out=rw(4), in0=rw(1), in1=rw(3))
                # cw_j = ej * rg ; cwsum = ssum * rg
                cwrow = p12.tile([1, J * P2W], bf16, tag="cwr", bufs=1,
                                 name="cwrow")
                for j in range(J):
                    nc.vector.tensor_mul(
                        out=cwrow[0:1, j * P2W:(j + 1) * P2W],
                        in0=ej[0:1, j * P2W:(j + 1) * P2W], in1=rw(4),
                    )
                for j in range(J):
                    nc.gpsimd.dma_start(
                        cw_dram[j:j + 1, csl2],
                        cwrow[0:1, j * P2W:(j + 1) * P2W],
                    )
                nc.vector.tensor_mul(out=rw(5), in0=rw(0), in1=rw(4))
                nc.vector.tensor_copy(cwsum_bf[0:1, csl2], rw(5))

        # ================= steady phases (scoped pools) =================
        SCW = cfg.get("SCW", MCW)       # hid-phase sub-chunk width
        NSC = MCW // SCW
        with tc.tile_pool(name="hresp", bufs=1) as hres, \
             tc.tile_pool(name="streamp", bufs=1) as stream, \
             tc.tile_pool(name="tmpp", bufs=1) as tmp:

            def msl(mc):
                return slice(mc * MCW, (mc + 1) * MCW)

            HGRP = [list(range(g, min(g + HGS, HB))) for g in range(0, HB, HGS)]
            DGRP = [list(range(g, min(g + DGS, DB))) for g in range(0, DB, DGS)]

            bounce = [
                dram.tile([D, RSW], bf16, name=f"bounce{g}") for g in range(NRS)
            ]
            rsout = [
                dram.tile([RSD, RSW], bf16, name=f"rsout{g}") for g in range(NRS)
            ]

            def a_or_m_phase(mc, which, m1b_t=None):
                src, wsrc = (huT, w1a) if which == "a" else (hmT, w1b)
                h_mc = hmc_tile(f"h_{which}{mc}")
                nc.sync.dma_start(
                    h_mc[:, :, :],
                    src.ap().rearrange("(o p) m -> p o m", p=128)[:, :, msl(mc)],
                )
                DBB = min(4, DB)
                for grp in HGRP:
                    gw = len(grp) * 128
                    psa = [psum_mm(f"ps{which}{i}") for i in range(len(grp))]
                    for dbb in range(0, DB, DBB):
                        w_t = w1t_tile("w1_t")
                        nc.sync.dma_start(
                            w_t[:, :DBB, :gw],
                            wsrc[dbb * 128:(dbb + DBB) * 128,
                                 grp[0] * 128: grp[0] * 128 + gw].rearrange(
                                     "(o p) h -> p o h", p=128),
                        )
                        for i in range(DBB):
                            db = dbb + i
                            for gi, hb in enumerate(grp):
                                nc.tensor.matmul(
                                    psa[gi][:, :MCW],
                                    w_t[:, i, gi * 128:(gi + 1) * 128],
                                    h_mc[:, db, :],
                                    start=db == 0, stop=db == DB - 1,
                                )
                    # psum -> SBUF casts on ScalarE (DVE is the busy engine)
                    for gi, hb in enumerate(grp):
                        if which == "a":
                            nc.scalar.activation(
                                A1T[:, hb, PAD + mc * MCW: PAD + (mc + 1) * MCW],
                                psa[gi][:, :MCW], AF.Identity,
                                bias=0.0, scale=1.0,
                            )
                        else:
                            for s in range(NSC):
                                nc.scalar.activation(
                                    m1b_t[s][:, hb, :],
                                    psa[gi][:, s * SCW:(s + 1) * SCW],
                                    AF.Identity,
                                    bias=b1_sb[:, hb:hb + 1], scale=1.0,
                                )

            def cwb_build(mc):
                # broadcast combine-weight rows to 128 partitions, early so
                # the K=1 matmuls slot into the A-phase and hid never waits
                cwsl = tmp.tile([1, J * MCW], bf16, tag="cwsl", bufs=1,
                                name="cwsl")
                for j in range(J):
                    nc.gpsimd.dma_start(
                        cwsl[0:1, j * MCW:(j + 1) * MCW],
                        cw_dram[j:j + 1, msl(mc)],
                    )
                cwbs = []
                for s in range(NSC):
                    cwb = tmp.tile([128, J, SCW], bf16, tag="cwb", bufs=3,
                                   name=f"cwb{mc}_{s}")
                    for j in range(J):
                        psb = psum_mm(f"psb{j}")
                        nc.tensor.matmul(
                            psb[:, :SCW], ones[0:1, :],
                            cwsl[0:1, j * MCW + s * SCW: j * MCW + (s + 1) * SCW],
                            start=True, stop=True,
                        )
                        nc.vector.tensor_copy(cwb[:, j, :], psb[:, :SCW])
                    cwbs.append(cwb)
                return cwbs

            def hid_sub(mc, s, m1b_t, hid_t, cwb):
                c0 = mc * MCW + s * SCW
                for hb in range(HB):
                    for j in range(J):
                        off = int(offs[j])
                        x_t = tmp.tile([128, SCW], bf16, tag="xt", bufs=2,
                                       name="x_t")
                        nc.vector.tensor_add(
                            out=x_t[:, :],
                            in0=A1T[:, hb, PAD + off + c0: PAD + off + c0 + SCW],
                            in1=m1b_t[:, hb, :],
                        )
                        g_t = tmp.tile([128, SCW], bf16, tag="gt", bufs=2,
                                       name="g_t")
                        nc.scalar.activation(
                            g_t[:, :], x_t[:, :], hid_af, bias=0.0, scale=1.0,
                        )
                        dst = hid_t[:, hb, s * SCW:(s + 1) * SCW]
                        if j == 0:
                            nc.vector.tensor_mul(
                                out=dst, in0=g_t[:, :], in1=cwb[:, j, :],
                            )
                        else:
                            nc.vector.tensor_mul(
                                out=g_t[:, :], in0=g_t[:, :], in1=cwb[:, j, :]
                            )
                            nc.vector.tensor_add(out=dst, in0=dst, in1=g_t[:, :])

            def w2_mc(mc, hid_t):
                c0 = mc * MCW
                g = c0 // RSW
                col0 = c0 - g * RSW
                HBB = min(4, HB)
                for grp in DGRP:
                    gw = len(grp) * 128
                    psd = [psum_mm(f"psd{i}") for i in range(len(grp))]
                    for hbb in range(0, HB, HBB):
                        w2_t = w2t_tile("w2_t")
                        nc.sync.dma_start(
                            w2_t[:, :HBB, :gw],
                            w2[hbb * 128:(hbb + HBB) * 128,
                               grp[0] * 128: grp[0] * 128 + gw].rearrange(
                                   "(o p) d -> p o d", p=128),
                        )
                        for i in range(HBB):
                            hb = hbb + i
                            for gi, db in enumerate(grp):
                                nc.tensor.matmul(
                                    psd[gi][:, :MCW],
                                    w2_t[:, i, gi * 128:(gi + 1) * 128],
                                    hid_t[:, hb, :],
                                    start=hb == 0, stop=False,
                                )
                    for gi, db in enumerate(grp):
                        nc.tensor.matmul(
                            psd[gi][:, :MCW],
                            b2_sb[0:1, db * 128:(db + 1) * 128],
                            cwsum_bf[0:1, c0:c0 + MCW],
                            start=False, stop=True,
                        )
                        d_t = tmp.tile([128, MCW], bf16, tag="dt", bufs=1,
                                       name="d_t")
                        # psum -> SBUF on ScalarE: keeps DVE free for hid work
                        nc.scalar.activation(
                            d_t[:, :], psd[gi][:, :MCW], AF.Identity,
                            bias=0.0, scale=1.0,
                        )
                        nc.sync.dma_start(
                            bounce[g][db * 128:(db + 1) * 128,
                                      col0:col0 + MCW],
                            d_t[:, :],
                        )

            def rs_maybe(mc):
                if (mc + 1) % max(1, RSW // MCW) == 0:
                    g = (mc * MCW) // RSW
                    nc.gpsimd.collective_compute(
                        "ReduceScatter",
                        mybir.AluOpType.add,
                        ins=[bounce[g].opt()],
                        outs=[rsout[g].opt()],
                        replica_groups=[list(range(NC))],
                    )
                    nc.gpsimd.dma_start(
                        outp[:, g * RSW:(g + 1) * RSW], rsout[g][:, :]
                    )

            m1b_store, hid_store, cwb_store = {}, {}, {}

            for mc in range(NMC):
                a_or_m_phase(mc, "a")
                cwb_store[mc] = cwb_build(mc)
                m1b_store[mc] = [
                    tmp.tile([128, HB, SCW], bf16, tag="m1b", bufs=3,
                             name=f"m1b_{mc}_{s}")
                    for s in range(NSC)
                ]
                a_or_m_phase(mc, "m", m1b_store[mc])
                hid_store[mc] = tmp.tile([128, HB, MCW], bf16, tag="hidct",
                                         bufs=2, name=f"hid_{mc}")
                if mc >= 1:
                    # previous chunk's last sub-chunk needed this A's halo
                    pm, ls = mc - 1, NSC - 1
                    hid_sub(pm, ls, m1b_store[pm][ls], hid_store[pm],
                            cwb_store[pm][ls])
                    w2_mc(pm, hid_store[pm])
                    rs_maybe(pm)
                    del m1b_store[pm], cwb_store[pm], hid_store[pm]
                for s in range(NSC - 1):
                    hid_sub(mc, s, m1b_store[mc][s], hid_store[mc],
                            cwb_store[mc][s])
            mc, ls = NMC - 1, NSC - 1
            hid_sub(mc, ls, m1b_store[mc][ls], hid_store[mc], cwb_store[mc][ls])
            w2_mc(mc, hid_store[mc])
            rs_maybe(mc)

    nc.finalize()
    return nc


# ----------------------------------------------------------------------------
# Host wrapper
# ----------------------------------------------------------------------------

def _prepare(inputs, cfg):
    import ml_dtypes
    BF16 = ml_dtypes.bfloat16
    D, H, M, U, DP, K = cfg["D"], cfg["H"], cfg["M"], cfg["U"], cfg["DP"], cfg["K"]
    HB, DPB = H // 128, DP // 128
    offs, valid = cfg["offs"], cfg["valid"]
    J = len(offs)
    QCW = cfg["QCW"]
    NQC = M // QCW

    h = np.asarray(inputs["h_L"], dtype=np.float32)[0]
    m_idx = np.asarray(inputs["mask_indices"]).astype(np.int64)
    u_idx = np.asarray(inputs["unmasked_indices"]).astype(np.int64)

    hmT = np.ascontiguousarray(h[m_idx].astype(BF16).T)
    huT = np.ascontiguousarray(h[u_idx].astype(BF16).T)
    wq = np.asarray(inputs["Wq"], np.float32).astype(BF16)
    wk = np.asarray(inputs["Wk"], np.float32).astype(BF16)
    wr = np.asarray(inputs["Wr"], np.float32).astype(BF16)
    bq = np.asarray(inputs["bq"], np.float32)
    bk = np.asarray(inputs["bk"], np.float32)
    brc = np.zeros((128, 1), np.float32)
    brc[:K, 0] = np.asarray(inputs["br"], np.float32)
    vm = np.ascontiguousarray(valid).astype(BF16)  # [J, M]

    W1 = np.asarray(inputs["W1"], np.float32)
    W2 = np.asarray(inputs["W2"], np.float32)
    b1 = np.asarray(inputs["b1"], np.float32)
    b2 = np.asarray(inputs["b2"], np.float32)

    DPBT = max(1, DP // 128)
    in_maps = []
    for c in range(cfg["NC"]):
        e = c % K
        dpb = c % DPBT
        dsl = slice(dpb * 128, (dpb + 1) * 128)
        sel = np.zeros((K, 1), np.float32)
        sel[e, 0] = 1.0
        in_maps.append({
            "hmT": hmT, "huT": huT,
            "w1a": np.ascontiguousarray(W1[e][:D]).astype(BF16),
            "w1b": np.ascontiguousarray(W1[e][D:]).astype(BF16),
            "w2": W2[e].astype(BF16),
            "wq": np.ascontiguousarray(wq[:, dsl]),
            "wk": np.ascontiguousarray(wk[:, dsl]),
            "wr": wr,
            "b1c": np.ascontiguousarray(b1[e].reshape(HB, 128).T),
            "b2r": b2[e].reshape(1, D).astype(BF16),
            "bqc": np.ascontiguousarray(bq[dsl].reshape(128, 1)),
            "bkc": np.ascontiguousarray(bk[dsl].reshape(128, 1)),
            "brc": brc,
            "esel": sel, "vmask": vm,
        })
    return in_maps, m_idx


def _run(cfg, in_maps, trace=False, sim=False):
    global LAST_RESULT
    key = cfg["key"]
    if key not in _GRAPH_CACHE:
        _GRAPH_CACHE[key] = build_graph(cfg)
    nc = _GRAPH_CACHE[key]
    if sim:
        from concourse import bass_interp
        s = bass_interp.MultiCoreSim(nc, cfg["NC"])
        for c in range(cfg["NC"]):
            for k, v in in_maps[c].items():
                s.cores[c].tensor(k)[:] = v
        s.simulate(check_with_hw=False)
        return [{"out": np.asarray(s.cores[c].mem_tensor("out"))}
                for c in range(cfg["NC"])]
    from concourse import bass_utils
    kw = {}
    if trace and os.environ.get("KERNEL_TRACE_DIR"):
        kw["tmpdir"] = os.environ["KERNEL_TRACE_DIR"]
    res = bass_utils.run_bass_kernel_spmd(
        nc, in_maps, core_ids=list(range(cfg["NC"])), trace=trace, **kw,
    )
    LAST_RESULT = res
    return res.results


def kernel_impl(inputs, D, K, L, M, U, DP, H, NC, MCW, QCW, NRS, sim=False,
                hid_act="Gelu", SCW=None):
    PMAX = M * 10

    m_idx = np.asarray(inputs["mask_indices"]).astype(np.int64)
    u_idx = np.asarray(inputs["unmasked_indices"]).astype(np.int64)
    r = int(np.asarray(inputs["range_r"]))

    offs, valid = build_tables(m_idx, u_idx, r, PMAX)
    J = len(offs)
    if J == 0:
        return np.zeros((1, L, D), np.float32)
    PAD = int(max(8, np.max(np.abs(offs))))
    PAD = (PAD + 7) // 8 * 8

    cfg = {
        "D": D, "H": H, "M": M, "U": U, "DP": DP, "K": K, "NC": NC,
        "offs": offs, "valid": valid, "PAD": PAD,
        "MCW": MCW, "QCW": QCW, "NRS": NRS, "hid_act": hid_act,
        "SCW": SCW or MCW,
        "key": (D, H, M, U, DP, K, NC, MCW, QCW, NRS, PAD, hid_act,
                SCW or MCW, tuple(offs.tolist())),
    }

    in_maps, m_idx = _prepare(inputs, cfg)
    results = _run(cfg, in_maps, trace=bool(os.environ.get("KERNEL_TRACE")),
                   sim=sim)

    deltaT = np.concatenate(
        [np.asarray(results[c]["out"], np.float32) for c in range(NC)], axis=0
    )  # [D, M]
    delta_md = deltaT.T  # [M, D]
    out = np.zeros((L, D), np.float32)
    if len(np.unique(m_idx)) == len(m_idx):
        out[m_idx] = delta_md
    else:
        np.add.at(out, m_idx, delta_md)
    return out[None]


def kernel(**inputs):
    return kernel_impl(
        inputs, D=4096, K=8, L=2048, M=1024, U=1024, DP=512, H=2048,
        NC=NCORES, MCW=512, QCW=512, NRS=2, SCW=256,
    )
